# revision 2
# baseline (speedup 1.0000x reference)
"""Multi-head attention (B=4, S=2048, D=512, H=8, E=64) on 8 TRN2 NeuronCores.

V3: fully software-pipelined PE-dense schedule.
  - host pre-transposes q/k/v; q/k cast to fp8-e4m3, fp8 DoubleRow q/k
    projections (K=256 per pass).
  - cross-ITERATION pipeline: cT ping-pongs between two buffers; the out
    projection of iteration i-1 runs as PE filler inside iteration i's
    attention pairs. Repeat loops emit two bodies (ping+pong) per For_i
    iteration; an epilogue out-projection drains the last body.
  - minimal front (group-0 projections + 2 v tiles), all other
    projections + out-proj spread as per-unit PE filler so ACT (exp) runs
    at ~90% duty behind a PE-bound pipeline.
  - normalization: bf16 reciprocal + raw cT evac (fast ot release), row
    broadcast via DRAM-bounce DMA on the ACT queue, in-place bf16
    2x-mode multiply.
  - v bias (and the softmax-denominator ones column) applied during the
    vh evacuation via a hoisted broadcast row; no K=1 bias matmuls.
PSUM: tag "st" 2x2 banks (S^T), "ot" 1x2 (PV accumulator),
      "proj" 2x1 (pq/pk/pv_main/yp), "pvo" shares proj tag.
"""

import numpy as np
import ml_dtypes

import concourse.bacc as bacc
import concourse.mybir as mybir
import concourse.tile as tile
from concourse import bass_utils

P = 128
D = 512
H = 8
E = 64
NG = H // 2
B_FULL, S_FULL = 4, 2048
N_CORES = 8
SQ = 1024
SK = 2048

WIDTHS = [65 for h in range(H)]
OFF = np.cumsum([0] + WIDTHS).tolist()
A = OFF[-1]            # 520

F32 = mybir.dt.float32
BF16 = mybir.dt.bfloat16
FP8 = mybir.dt.float8e4
DR = mybir.MatmulPerfMode.DoubleRow


def build_nc(sq=SQ, sk=SK, cfg=None, repeat=1, phases=4,
             stbufs=2, otbufs=1, projbufs=2, exbufs=6, unroll=1,
             loop_kwargs=None):
    sqt, skt, ndt = sq // P, sk // P, D // P
    qcs = 512
    nqc = sq // qcs
    kcs = 512
    nkc = sk // kcs

    nc = bacc.Bacc("TRN2", target_bir_lowering=False, debug=False)
    di = {}
    for name, shape, dt in [
        ("qT_loc", [D, sq], FP8), ("kT_loc", [D, sk], FP8), ("vT_loc", [D, sk], BF16),
        ("Wqg", [NG, D, P], FP8), ("Wkg", [NG, D, P], FP8),
        ("bqg", [P, NG], F32), ("bkg", [P, NG], F32),
        ("Wv_aug", [D, A], BF16), ("bv_row", [1, A], BF16), ("WoTh", [64, H, D], BF16),
    ]:
        di[name] = nc.dram_tensor(name, shape, dt, kind="ExternalInput").ap()
    den_t = nc.dram_tensor("den_s", [H, sq], BF16, kind="Internal").ap()
    y_t = nc.dram_tensor("y_loc", [sq, D], F32, kind="ExternalOutput").ap()

    from contextlib import ExitStack
    with tile.TileContext(nc) as tc, ExitStack() as top:
        pers = top.enter_context(tc.tile_pool(name="pers", bufs=1))
        wq = pers.tile([P, ndt, NG, P], FP8, name="wq")
        wk = pers.tile([P, ndt, NG, P], FP8, name="wk")
        wv = pers.tile([P, ndt, A], BF16, name="wv")
        wo = pers.tile([64, H, D], BF16, name="wo")
        bq_sb = pers.tile([P, NG], F32, name="bq_sb")
        bk_sb = pers.tile([P, NG], F32, name="bk_sb")
        bv_bc = pers.tile([P, A], BF16, name="bv_bc")
        qhp = [pers.tile([P, 2, sq], BF16, name=f"qhp_{g}") for g in range(NG)]
        khp = [pers.tile([P, 2, sk], BF16, name=f"khp_{g}") for g in range(NG)]
        rc_sb = pers.tile([P, sq], BF16, name="rc_sb")
        cTab = [pers.tile([64, H, sq], BF16, name=f"cT_{i}") for i in range(2)]
        vhp = top.enter_context(tc.tile_pool(name="vhp", bufs=2))
        rcbp = top.enter_context(tc.tile_pool(name="rcbp", bufs=2))

        # ---- hoisted setup: weights + constants (iteration-invariant) ----
        for g in range(NG):
            nc.vector.memset(qhp[g][:], 0.0)
            nc.vector.memset(khp[g][:], 0.0)
        nc.vector.memset(cTab[0][:], 0.0)
        nc.vector.memset(cTab[1][:], 0.0)
        for g in range(NG):
            nc.sync.dma_start(wq[:, :, g, :],
                              di["Wqg"][g].rearrange("(do di) m -> di do m", di=P))
            nc.sync.dma_start(wk[:, :, g, :],
                              di["Wkg"][g].rearrange("(do di) m -> di do m", di=P))
        nc.sync.dma_start(wv[:], di["Wv_aug"].rearrange("(do di) m -> di do m", di=P))
        nc.gpsimd.dma_start(wo[:], di["WoTh"])
        nc.sync.dma_start(bq_sb[:], di["bqg"])
        nc.sync.dma_start(bk_sb[:], di["bkg"])
        nc.sync.dma_start(bv_bc[:], di["bv_row"].broadcast_to([P, A]))

        def make_outproj(ps, sb, ct):
            def outproj(qt):
                yp = ps.tile([P, 512], F32, tag="proj", name=f"yp_{qt}",
                             bufs=projbufs)
                for h in range(H):
                    nc.tensor.matmul(yp[:], ct[:, h, qt * P:(qt + 1) * P],
                                     wo[:, h, :], start=(h == 0),
                                     stop=(h == H - 1))
                ys = sb.tile([P, 512], F32, tag="y", name=f"ys_{qt}", bufs=3)
                nc.vector.tensor_copy(ys[:], yp[:])
                nc.gpsimd.dma_start(y_t[qt * P:(qt + 1) * P, :], ys[:])
            return outproj

        def body(cur, prv):
            cT, cTp = cTab[cur], cTab[prv]
            with ExitStack() as es:
                xT = es.enter_context(tc.tile_pool(name="xT", bufs=1))
                ps = es.enter_context(tc.tile_pool(name="ps", bufs=1, space="PSUM"))
                sb = es.enter_context(tc.tile_pool(name="sbw", bufs=1))

                qT = xT.tile([P, ndt, sq], FP8, name="qT")
                kT = xT.tile([P, ndt, sk], FP8, name="kT")
                vT = xT.tile([P, ndt, sk], BF16, name="vT")
                vh = vhp.tile([P, skt, A], BF16, tag="vh", name="vh", bufs=2)
                nc.sync.dma_start(qT[:], di["qT_loc"].rearrange("(t p) s -> p t s", p=P))
                nc.sync.dma_start(kT[:], di["kT_loc"].rearrange("(t p) s -> p t s", p=P))
                nc.sync.dma_start(vT[:], di["vT_loc"].rearrange("(t p) s -> p t s", p=P))

                if phases < 2:
                    return

                def qproj(g, c):
                    pq = ps.tile([P, qcs], F32, tag="proj", name=f"pq_{g}_{c}",
                                 bufs=projbufs)
                    for j in range(ndt // 2):
                        nc.tensor.matmul(
                            pq[:], wq[:, 2 * j:2 * j + 2, g, :],
                            qT[:, 2 * j:2 * j + 2, c * qcs:(c + 1) * qcs],
                            start=(j == 0), stop=(j == ndt // 2 - 1),
                            perf_mode=DR)
                    sl = slice(c * qcs, (c + 1) * qcs)
                    nc.vector.tensor_scalar_add(
                        qhp[g][0:64, 0, sl], pq[0:64, :], bq_sb[0:64, g:g + 1])
                    nc.vector.tensor_scalar_add(
                        qhp[g][64:128, 1, sl], pq[64:128, :],
                        bq_sb[64:128, g:g + 1])

                def kproj(g, c):
                    pk = ps.tile([P, kcs], F32, tag="proj", name=f"pk_{g}_{c}",
                                 bufs=projbufs)
                    for j in range(ndt // 2):
                        nc.tensor.matmul(
                            pk[:], wk[:, 2 * j:2 * j + 2, g, :],
                            kT[:, 2 * j:2 * j + 2, c * kcs:(c + 1) * kcs],
                            start=(j == 0), stop=(j == ndt // 2 - 1),
                            perf_mode=DR)
                    sl = slice(c * kcs, (c + 1) * kcs)
                    nc.vector.tensor_scalar_add(
                        khp[g][0:64, 0, sl], pk[0:64, :], bk_sb[0:64, g:g + 1])
                    nc.vector.tensor_scalar_add(
                        khp[g][64:128, 1, sl], pk[64:128, :],
                        bk_sb[64:128, g:g + 1])

                def vproj(tt):
                    pv = ps.tile([P, 512], F32, tag="proj", name=f"pv_{tt}",
                                 bufs=projbufs)
                    po = ps.tile([P, A - 512], F32, tag="proj", name=f"po_{tt}",
                                 bufs=projbufs)
                    for t in range(ndt):
                        nc.tensor.matmul(pv[:], vT[:, t, tt * P:(tt + 1) * P],
                                         wv[:, t, 0:512], start=(t == 0),
                                         stop=(t == ndt - 1))
                        nc.tensor.matmul(po[:], vT[:, t, tt * P:(tt + 1) * P],
                                         wv[:, t, 512:A], start=(t == 0),
                                         stop=(t == ndt - 1))
                    nc.vector.tensor_tensor(vh[:, tt, 0:512], pv[:],
                                            bv_bc[:, 0:512], mybir.AluOpType.add)
                    nc.vector.tensor_tensor(vh[:, tt, 512:A], po[:],
                                            bv_bc[:, 512:A], mybir.AluOpType.add)

                outproj = make_outproj(ps, sb, cTp)

                for c in range(nqc):
                    qproj(0, c)
                for c in range(nkc):
                    kproj(0, c)
                if phases < 3:
                    for g in range(1, NG):
                        for c in range(nqc):
                            qproj(g, c)
                    for c in range(nkc):
                        for g in range(1, NG):
                            kproj(g, c)
                    for tt in range(skt):
                        vproj(tt)
                    return

                for tt in range(2):
                    vproj(tt)

                def s_unit(h, tt):
                    g, j = h // 2, h % 2
                    st = ps.tile([P, sq], F32, tag="st", name=f"st_{h}_{tt}",
                                 bufs=stbufs)
                    for c in range(nqc):
                        nc.tensor.matmul(
                            st[:, c * qcs:(c + 1) * qcs],
                            khp[g][:, j, tt * P:(tt + 1) * P],
                            qhp[g][:, j, c * qcs:(c + 1) * qcs],
                            start=True, stop=True)
                    ex = sb.tile([P, sq], BF16, tag="ex", name=f"ex_{h}_{tt}",
                                 bufs=exbufs)
                    nc.scalar.activation(ex[:], st[:],
                                         mybir.ActivationFunctionType.Exp,
                                         scale=0.125)
                    return ex

                def pv_unit(h, tt, ot, ex):
                    for c in range(nqc):
                        nc.tensor.matmul(
                            ot[0:65, c * qcs:(c + 1) * qcs],
                            vh[:, tt, OFF[h]:OFF[h] + 65],
                            ex[:, c * qcs:(c + 1) * qcs],
                            start=(tt == 0), stop=(tt == skt - 1))

                def norm(h, ot, cT):
                    with nc.allow_low_precision("softmax denom recip in bf16"):
                        nc.vector.reciprocal(rc_sb[64:65, :], ot[64:65, :])
                        nc.vector.tensor_copy(cT[:, h, :], ot[0:64, :])
                    nc.scalar.dma_start(den_t[h:h + 1, :], rc_sb[64:65, :])
                    rcb = rcbp.tile([64, sq], BF16, tag="rcb", name=f"rcb_{h}",
                                    bufs=2)
                    nc.scalar.dma_start(rcb[:],
                                        den_t[h:h + 1, :].broadcast_to([64, sq]))
                    nc.vector.tensor_tensor(cT[:, h, :], cT[:, h, :],
                                            rcb[:], mybir.AluOpType.mult)

                for h in range(H):
                    g = h // 2
                    ot = ps.tile([P, sq], F32, tag="ot", name=f"ot_{h}",
                                 bufs=otbufs)
                    for tt in range(skt):
                        # per-unit PE filler keeps PE the bottleneck while
                        # ACT streams exps behind it
                        if h == 0 and tt + 2 < skt:
                            vproj(tt + 2)
                        if h in (1, 3, 5) and tt % 4 == 1:
                            kproj(g + 1, tt // 4)
                        if h in (1, 3, 5) and tt in (3, 11):
                            qproj(g + 1, tt // 8)
                        if phases >= 4 and h in (2, 3, 4, 5) and tt in (2, 10):
                            outproj((h - 2) * 2 + tt // 8)
                        ex = s_unit(h, tt)
                        pv_unit(h, tt, ot, ex)
                    norm(h, ot, cT)

        def epilogue(cur):
            with ExitStack() as es:
                ps = es.enter_context(tc.tile_pool(name="pse", bufs=1, space="PSUM"))
                sb = es.enter_context(tc.tile_pool(name="sbe", bufs=1))
                outproj = make_outproj(ps, sb, cTab[cur])
                for qt in range(sqt):
                    outproj(qt)

        if repeat == 1:
            last = 0
            for i in range(unroll):
                body(i % 2, 1 - i % 2)
                last = i % 2
            if phases >= 4:
                epilogue(last)
        else:
            lk = dict(loop_kwargs) if loop_kwargs else {
                "staggered_reset": True, "loop_bodies": 4}
            nb = lk.pop("loop_bodies", 2)
            if repeat % nb != 0:
                nb = 2
            assert repeat % nb == 0
            with tc.For_i(0, repeat // nb, 1, **lk):
                for i in range(nb):
                    body(i % 2, 1 - i % 2)
            if phases >= 4:
                epilogue(1)

    nc.compile()
    return nc


def host_pack(Wq, bq, Wk, bk, Wv, bv, Wo):
    Wq, bq, Wk, bk, Wv, bv, Wo = [np.asarray(x, np.float32) for x in
                                  (Wq, bq, Wk, bk, Wv, bv, Wo)]
    bf = ml_dtypes.bfloat16
    f8 = ml_dtypes.float8_e4m3
    Wqg = np.ascontiguousarray(np.stack(
        [np.concatenate([Wq[2 * g], Wq[2 * g + 1]], axis=1) for g in range(NG)])).astype(f8)
    Wkg = np.ascontiguousarray(np.stack(
        [np.concatenate([Wk[2 * g], Wk[2 * g + 1]], axis=1) for g in range(NG)])).astype(f8)
    bqg = np.ascontiguousarray(np.stack(
        [np.concatenate([bq[2 * g], bq[2 * g + 1]]) for g in range(NG)], axis=1))
    bkg = np.ascontiguousarray(np.stack(
        [np.concatenate([bk[2 * g], bk[2 * g + 1]]) for g in range(NG)], axis=1))
    Wv_aug = np.zeros((D, A), np.float32)
    bv_row = np.zeros((1, A), np.float32)
    for h in range(H):
        o = OFF[h]
        Wv_aug[:, o:o + 64] = Wv[h]
        bv_row[0, o:o + 64] = bv[h]
        bv_row[0, o + 64] = 1.0
    WoTh = np.ascontiguousarray(Wo.T.reshape(H, 64, D).transpose(1, 0, 2)).astype(bf)
    return {"Wqg": Wqg, "Wkg": Wkg, "bqg": bqg, "bkg": bkg,
            "Wv_aug": Wv_aug.astype(bf), "bv_row": bv_row.astype(bf),
            "WoTh": WoTh}


def make_core_input(q_loc, k_loc, v_loc, packed):
    bf = ml_dtypes.bfloat16
    f8 = ml_dtypes.float8_e4m3
    return {
        "qT_loc": np.ascontiguousarray(np.asarray(q_loc).T).astype(f8),
        "kT_loc": np.ascontiguousarray(np.asarray(k_loc).T).astype(f8),
        "vT_loc": np.ascontiguousarray(np.asarray(v_loc).T).astype(bf),
        **packed,
    }


_NC_CACHE = {}


def _get_nc(repeat=1):
    if repeat not in _NC_CACHE:
        _NC_CACHE[repeat] = build_nc(repeat=repeat)
    return _NC_CACHE[repeat]


def make_in_maps(q, k, v, Wq, bq, Wk, bk, Wv, bv, Wo):
    q, k, v = [np.asarray(x, np.float32) for x in (q, k, v)]
    packed = host_pack(Wq, bq, Wk, bk, Wv, bv, Wo)
    return [
        make_core_input(q[c // 2, (c % 2) * SQ:(c % 2) * SQ + SQ],
                        k[c // 2], v[c // 2], packed)
        for c in range(N_CORES)
    ]


def assemble(results):
    out = np.empty((B_FULL, S_FULL, D), np.float32)
    for c in range(N_CORES):
        b, qlo = c // 2, (c % 2) * SQ
        out[b, qlo:qlo + SQ] = results[c]["y_loc"]
    return out


def kernel(q, k, v, Wq, bq, Wk, bk, Wv, bv, Wo):
    nc = _get_nc(repeat=1)
    in_maps = make_in_maps(q, k, v, Wq, bq, Wk, bk, Wv, bv, Wo)
    res = bass_utils.run_bass_kernel_spmd(nc, in_maps, core_ids=list(range(N_CORES)))
    return assemble(res.results)
